# revision 50
# baseline (speedup 1.0000x reference)
"""MuSc (Mutual Scoring) Trainium2 kernel — v3 (truncated rounds + wide rescue).

Problem: nn_BatchMuSc — Z:[16,1369,1024] patch features, cls_tokens:[16,1024].
MSM: for each image i, per-patch score = mean of the 4 smallest per-image
min-distances (excluding self). Then image scores -> min-max norm -> MMO over
cls-token similarity.

Key insight: phase 1's ONLY job is selecting rescue candidates — the final
image scores come entirely from the exact fp16 rescue (phase 2). So phase 1
can be aggressively approximate:
  - RUSE=2 of the 15 pair-rounds are computed (each image scored against 2
    partners, statistic = mean of those 2 min-distances). Validated on host
    with exact distances: top-32 candidate selection recovers the argmax
    patch with 0/1280 failures even at 2x the empirical fp8 noise level
    (sigma_emp = 4.15 on d^2, worst model rank 23); on-device worst-case
    argmax rank is 15 of 32.
  - SYMMETRY: each unordered image pair {a,b} is computed ONCE as a
    [1408q x 1408r] block and reduced along BOTH axes: free-axis max ->
    a's patches vs b; partition-axis max (GpSimd all-reduce) -> b's patches
    vs a. 2 rounds x 8 cores = 16 pairs (1-factorization of K16).
  - fp8 e4m3 + DoubleRow matmuls (K=256 per MM): PE runs at the 157 TF/s fp8
    roofline, ~98% window utilization.
  - Both norms ride in the stream: ref-side -0.5|r|^2 as base-{64,8,1} fp8
    digit rows (feature rows 1021-1023 repurposed), query-side -0.5|q|^2 as
    the f32 ACT bias during the psum->f16 staging copy. So staged s2 =
    q.r - 0.5|r|^2 - 0.5|q|^2 and min d^2 = -2*max on both axes.
  - Per query block: 12 DR matmuls -> 2 ACT staging copies (psum->s2 f16,
    +bias) -> DVE acc=max(acc,s2) (partition side) -> DVE tensor_reduce
    (free side). Reading the staged f16 instead of f32 PSUM keeps DVE off
    the critical path.
  - DMA: k-subtile-major contiguous DRAM slabs; round-0 subtile split lets
    the first matmul start after ~700KB instead of 2.8MB.
  - Phase 2 (exact rescue): top-32 patches per image = 512 candidates,
    scored against ALL 15 other images at fp16, sharded over ref images
    (2 per core). Refs are the stationary operand (psum[ref, cand]); the
    candidates stream as one 512-wide rhs; the exact f32 ref norm rides as
    the ACT bias during psum->f32 staging; DVE maxes across j-blocks and
    the host does the final partition max. No aug matmuls, no on-device
    partition reduce. Host does the tiny 4-smallest/MMO tail in f64.
"""

import os
import numpy as np
import ml_dtypes

N = 16            # images
L = 1369          # patches per image
C = 1024          # feature dim
NCORES = 8
LP = 1408         # padded patches (11 * 128)
NQB = 11          # query blocks of 128
KCH = 8           # contraction chunks of 128
NSP = KCH // 2    # k-subtiles of 2 chunks (one fp8-DR matmul's contraction)
R = 15            # full pair-round count (1-factorization of K16)
RUSE = 2          # rounds actually computed: each image scored against 2
                  # partners; candidate top-32 is then exact-rescued against
                  # all 15 (see module docstring for the validation)
CHUNKS = [(0, 512), (512, 512), (1024, 352)]   # covers 1369 real + 7 pad cols
LE = 1376         # elementwise width (mult of 32)
PAD_VAL = 2.0     # pad-row feature value; pad distances are huge -> never win
BIG = 3.0e38
NTERM = 1         # rescue query terms (qh only; fp16 precision suffices)
NT2 = KCH * NTERM  # rescue lhsT slots
NCAND = 32        # rescued candidates per image (512 streamed columns)
NSLAB = N * NCAND // 128

_CACHE = {}


def _pair_schedule():
    """sched[r][c] = (a, b): round-robin 1-factorization of K16."""
    sched = []
    for r in range(R):
        pairs = [(15, r)]
        for k in range(1, 8):
            pairs.append(((r + k) % R, (r - k) % R))
        sched.append(pairs)
    return sched


def _build1(fp8=True, rounds=R):
    import concourse.bacc as bacc
    import concourse.tile as tile
    from concourse import mybir, bass_isa

    f16 = mybir.dt.float16
    f32 = mybir.dt.float32
    dt_z = mybir.dt.float8e4 if fp8 else f16
    Alu = mybir.AluOpType
    Copy = mybir.ActivationFunctionType.Copy
    DR = mybir.MatmulPerfMode.DoubleRow

    nc = bacc.Bacc("TRN2", target_bir_lowering=False, debug=False)

    # k-subtile-major DRAM layout: zta[r, s] is one contiguous [128, 2, LP] slab
    zta = nc.dram_tensor("zta", [rounds, NSP, 128, 2, LP], dt_z, kind="ExternalInput").ap()
    ztb = nc.dram_tensor("ztb", [rounds, NSP, 128, 2, LP], dt_z, kind="ExternalInput").ap()
    qna = nc.dram_tensor("qna", [rounds, 128, NQB], f32, kind="ExternalInput").ap()
    outf = nc.dram_tensor("outf", [rounds, 128, NQB], f32, kind="ExternalOutput").ap()
    outp = nc.dram_tensor("outp", [rounds, 1, LE], f32, kind="ExternalOutput").ap()
    # last round's raw accumulator: host does its partition max (saves the
    # exposed 4.8us GpSimd all-reduce on the tail)
    acco = nc.dram_tensor("acco", [128, LE], f16, kind="ExternalOutput").ap()

    Identity = mybir.ActivationFunctionType.Identity

    with tile.TileContext(nc) as tc:
        with (
            tc.tile_pool(name="zpool", bufs=2) as zpool,
            tc.tile_pool(name="qnpool", bufs=2) as qnpool,
            tc.tile_pool(name="accpool", bufs=2) as accpool,
            tc.tile_pool(name="s2pool", bufs=2) as s2pool,
            tc.tile_pool(name="outfpool", bufs=2) as outfpool,
            tc.tile_pool(name="prpool", bufs=2) as prpool,
            tc.tile_pool(name="psumA", bufs=3, space="PSUM") as psumA,
            tc.tile_pool(name="psumB", bufs=2, space="PSUM") as psumB,
        ):
            for r in range(rounds):
                # per-ksub DMA split: the first matmul only needs subtile 0
                # of both operands (~700KB) instead of the full 2.8MB
                qn = qnpool.tile([128, NQB], f32, name="qn", tag="qn")
                nc.sync.dma_start(qn[:], qna[r])
                za_s, zb_s = [], []
                for s in range(NSP):
                    ta = zpool.tile([128, 2, LP], dt_z, name=f"za{s}", tag=f"za{s}")
                    nc.sync.dma_start(ta[:], zta[r, s])
                    za_s.append(ta)
                    tb = zpool.tile([128, 2, LP], dt_z, name=f"zb{s}", tag=f"zb{s}")
                    nc.sync.dma_start(tb[:], ztb[r, s])
                    zb_s.append(tb)

                acc = accpool.tile([128, LE], f16, name="acc", tag="acc")
                outf_t = outfpool.tile([128, NQB], f32, name="outf_t", tag="outf_t")

                for qb in range(NQB):
                    ptA = psumA.tile([128, 1024], f32, name="ptA", tag="ptA")
                    ptB = psumB.tile([128, 512], f32, name="ptB", tag="ptB")
                    for ci, (c0, w) in enumerate(CHUNKS):
                        dst = ptA[:, c0:c0 + w] if ci < 2 else ptB[:, :w]
                        if fp8:
                            for kp in range(NSP):
                                nc.tensor.matmul(
                                    dst,
                                    lhsT=za_s[kp][:, :, qb * 128:(qb + 1) * 128],
                                    rhs=zb_s[kp][:, :, c0:c0 + w],
                                    start=(kp == 0),
                                    stop=(kp == NSP - 1),
                                    perf_mode=DR,
                                )
                        else:
                            for k in range(KCH):
                                nc.tensor.matmul(
                                    dst,
                                    lhsT=za_s[k // 2][:, k % 2, qb * 128:(qb + 1) * 128],
                                    rhs=zb_s[k // 2][:, k % 2, c0:c0 + w],
                                    start=(k == 0),
                                    stop=(k == KCH - 1),
                                )
                    # partition side staging: s2 = psum + (-0.5|q|^2), fp16;
                    # qb 0 writes the accumulator directly
                    if qb == 0:
                        s2 = acc
                    else:
                        s2 = s2pool.tile([128, LE], f16, name="s2", tag="s2")
                    nc.scalar.activation(
                        s2[:, :1024], ptA[:, :1024], Identity,
                        bias=qn[:, qb:qb + 1], scale=1.0)
                    nc.scalar.activation(
                        s2[:, 1024:LE], ptB[:, :352], Identity,
                        bias=qn[:, qb:qb + 1], scale=1.0)
                    # accmax first so the final round's partition reduce can
                    # start while the last free-side reduce still runs
                    if qb > 0:
                        nc.vector.tensor_tensor(
                            acc[:, :LE], acc[:, :LE], s2[:, :LE], op=Alu.max)
                    # free side: max over refs from the f16 staged copy; outf
                    # includes the -0.5|q|^2 bias, so host uses -2*outf
                    nc.vector.tensor_reduce(
                        outf_t[:, qb:qb + 1], s2[:, :LE],
                        axis=mybir.AxisListType.X, op=Alu.max)

                if r < rounds - 1:
                    pr = prpool.tile([128, LE], f32, name="pr", tag="pr")
                    nc.gpsimd.partition_all_reduce(
                        pr[:], acc[:, :LE], channels=128,
                        reduce_op=bass_isa.ReduceOp.max)
                    nc.sync.dma_start(outp[r], pr[0:1, :])
                else:
                    nc.sync.dma_start(acco[:], acc[:, :LE])
                nc.sync.dma_start(outf[r], outf_t[:])
    nc.compile()
    return nc


NC_TOT = N * NCAND          # 256 rescue candidates
WAVES = [(0, 6), (6, NQB)]  # j-blocks per PSUM wave (6 + 5 one-bank tiles)


def _build2():
    """Exact rescue v2: refs stationary, candidates streamed.

    Per (pos, j-block): psum[ref, cand] = sum_k rk[k][:, jblock].T @ qct[k]
    (fp16, K=1024 over 8 matmuls, 256 streamed cols each). The exact ref norm
    -0.5|r|^2 rides as the per-partition f32 ACT bias during psum->SBUF f32
    staging; DVE maxes the staged blocks across j; the host does the final
    partition max (no on-device partition reduce, no aug matmuls). k-outer
    loop within each wave so the first matmuls need only qct[0]+rk[0]
    (~425KB) instead of the full 2.9MB."""
    import concourse.bacc as bacc
    import concourse.tile as tile
    from concourse import mybir

    f16 = mybir.dt.float16
    f32 = mybir.dt.float32
    Alu = mybir.AluOpType
    Identity = mybir.ActivationFunctionType.Identity

    nc = bacc.Bacc("TRN2", target_bir_lowering=False, debug=False)
    # two contiguous k-half candidate tensors (4KB rows each; finer per-k
    # tiles would DMA tiny descriptors and clog the queues ahead of the
    # refs, one full tile would delay the first matmul by ~1.5us)
    qc = nc.dram_tensor("qc", [2, 128, KCH // 2, NC_TOT], f16, kind="ExternalInput").ap()
    rh = nc.dram_tensor("rh", [2, KCH, 128, LP], f16, kind="ExternalInput").ap()
    rnb = nc.dram_tensor("rnb", [2, 128, NQB], f32, kind="ExternalInput").ap()
    acco2 = nc.dram_tensor("acco2", [2, 128, NC_TOT], f32, kind="ExternalOutput").ap()

    with tile.TileContext(nc) as tc:
        with (
            tc.tile_pool(name="qpool2", bufs=1) as qpool2,
            tc.tile_pool(name="ref2", bufs=2) as ref2,
            tc.tile_pool(name="rnpool", bufs=2) as rnpool,
            tc.tile_pool(name="acc2pool", bufs=2) as acc2pool,
            tc.tile_pool(name="s3pool", bufs=3) as s3pool,
            tc.tile_pool(name="ps2", bufs=6, space="PSUM") as ps2,
        ):
            qct = [None] * KCH

            def _load_qhalf(h):
                qtile = qpool2.tile([128, KCH // 2, NC_TOT], f16,
                                    name=f"qtile{h}", tag=f"qtile{h}")
                nc.sync.dma_start(qtile[:], qc[h])
                for kk_ in range(KCH // 2):
                    qct[h * (KCH // 2) + kk_] = qtile[:, kk_, :]

            _load_qhalf(0)
            for pos in range(2):
                rnbt = rnpool.tile([128, NQB], f32, name="rnbt", tag="rnbt")
                nc.sync.dma_start(rnbt[:], rnb[pos])
                rkt = []
                for k in range(KCH):
                    t_ = ref2.tile([128, LP], f16, name=f"rk{k}", tag=f"rk{k}")
                    nc.sync.dma_start(t_[:], rh[pos, k])
                    rkt.append(t_)
                    if pos == 0 and k == 0:
                        # second candidate half rides behind the first ref
                        # tile: the k=0 matmuls only need half 0
                        _load_qhalf(1)

                acc2 = acc2pool.tile([128, NC_TOT], f32, name="acc2", tag="acc2")
                for w0, w1 in WAVES:
                    pt = {}
                    for j in range(w0, w1):
                        # one full bank per j (256 f32 used, bank-aligned dst)
                        pt[j] = ps2.tile([128, 512], f32, name=f"pt{j}", tag="pt")
                    for k in range(KCH):
                        for j in range(w0, w1):
                            nc.tensor.matmul(
                                pt[j][:, :NC_TOT],
                                lhsT=rkt[k][:, j * 128:(j + 1) * 128],
                                rhs=qct[k][:],
                                start=(k == 0),
                                stop=(k == KCH - 1),
                            )
                    for j in range(w0, w1):
                        if j == 0:
                            s3 = acc2
                        else:
                            s3 = s3pool.tile([128, NC_TOT], f32, name="s3", tag="s3")
                        nc.scalar.activation(
                            s3[:, :NC_TOT], pt[j][:, :NC_TOT], Identity,
                            bias=rnbt[:, j:j + 1], scale=1.0)
                        if j > 0:
                            nc.vector.tensor_tensor(
                                acc2[:], acc2[:], s3[:, :NC_TOT], op=Alu.max)
                nc.sync.dma_start(acco2[pos], acc2[:])
    nc.compile()
    return nc


DIGIT_SCALES = (64.0, 8.0, 1.0)


def _digit_rows(v):
    """Decompose v (~[-2100, -400]) into base-{64,8,1} rows, last row e4m3."""
    d1 = np.round(v / 64.0)
    r1 = v - 64.0 * d1
    d2 = np.round(r1 / 8.0)
    d3 = r1 - 8.0 * d2
    return d1, d2, d3


def _host_prep(Z, fp8=True):
    """Quantized transposed tiles (a/b variants) + exact norms + qn bias.

    Feature rows 1021-1023 (p=125..127 of k-chunk 7) are repurposed:
    a-variant (lhsT) holds the constants {64, 8, 1}; b-variant (rhs) holds
    the base-{64,8,1} digit rows of -0.5|r|^2, so the DR stream itself
    computes q.r(1021 feats) - 0.5|r|^2.
    """
    Zp = np.full((N, LP, C), PAD_VAL, dtype=np.float32)
    Zp[:, :L, :] = Z
    qdt = ml_dtypes.float8_e4m3 if fp8 else np.float16
    Zq = Zp.astype(qdt)
    # [img, p, k, r] = Zq[img, r, 128k+p]
    zt = np.ascontiguousarray(Zq.reshape(N, LP, KCH, 128).transpose(0, 3, 2, 1))
    nrm = (Zp.astype(np.float64) ** 2).sum(-1)          # [N, LP] exact full norm
    zta = zt.copy()
    for j, s in enumerate(DIGIT_SCALES):
        zta[:, 125 + j, 7, :] = qdt(s)
    ztb = zt
    d1, d2, d3 = _digit_rows(-0.5 * nrm)
    ztb[:, 125, 7, :] = d1.astype(qdt)
    ztb[:, 126, 7, :] = d2.astype(qdt)
    ztb[:, 127, 7, :] = d3.astype(qdt)
    qna = np.ascontiguousarray(
        (-0.5 * nrm).astype(np.float32).reshape(N, NQB, 128).transpose(0, 2, 1))

    def ksub_major(zt_):
        # [img, p, k, l] -> [img, s, p, j, l] with k = 2s+j (contiguous
        # per-(img,s) slabs for large-descriptor DMA)
        t = zt_.transpose(0, 2, 1, 3).reshape(N, NSP, 2, 128, LP)
        return np.ascontiguousarray(t.transpose(0, 1, 3, 2, 4))

    return ksub_major(zta), ksub_major(ztb), nrm, qna


def _host_prep2(Z):
    """Rescue ref data: fp16 refs (k-major contiguous slabs) + exact f32
    -0.5|r|^2 bias in phase-1 qna layout ([img, p, j] for ref patch 128j+p)."""
    Zp = np.full((N, LP, C), PAD_VAL, dtype=np.float32)
    Zp[:, :L, :] = Z
    Zh = Zp.astype(np.float16)
    # [img, k, p, l] = Zh[img, l, 128k+p]  (rh[pos, k] contiguous)
    rh = np.ascontiguousarray(Zh.reshape(N, LP, KCH, 128).transpose(0, 2, 3, 1))
    nrm = (Zp.astype(np.float64) ** 2).sum(-1)
    rnb = np.ascontiguousarray(
        (-0.5 * nrm).astype(np.float32).reshape(N, NQB, 128).transpose(0, 2, 1))
    return rh, rnb


def _run_with_retry(nc, in_maps, trace, attempts=3):
    import time
    import traceback
    import concourse.bass_utils as bass_utils

    import jax
    jax.devices()   # force PJRT backend init before the NTFF profile hook

    for a in range(attempts):
        try:
            return bass_utils.run_bass_kernel_spmd(
                nc, in_maps, core_ids=list(range(NCORES)), trace=trace)
        except Exception:
            traceback.print_exc()
            if a == attempts - 1:
                raise
            time.sleep(5)


def kernel(Z, cls_tokens):
    Z = np.asarray(Z, dtype=np.float32)
    cls_tokens = np.asarray(cls_tokens)
    fp8 = bool(int(os.environ.get("KERNEL_FP8", "1")))
    trace = bool(int(os.environ.get("KERNEL_TRACE", "0")))

    if "nc1" not in _CACHE:
        _CACHE["nc1"] = _build1(fp8=fp8, rounds=RUSE)
    nc1 = _CACHE["nc1"]

    zta_all, ztb_all, nrm, qna = _host_prep(Z, fp8=fp8)
    sched = _pair_schedule()

    in_maps = []
    for c in range(NCORES):
        aa = [sched[r][c][0] for r in range(RUSE)]
        bb = [sched[r][c][1] for r in range(RUSE)]
        in_maps.append({
            "zta": np.ascontiguousarray(zta_all[aa]),
            "ztb": np.ascontiguousarray(ztb_all[bb]),
            "qna": np.ascontiguousarray(qna[aa]),
        })

    res = _run_with_retry(nc1, in_maps, trace)
    _CACHE["last_results"] = res

    # assemble per-patch min-d^2 matrix [img, patch, other-img]
    # (only RUSE of 15 partners computed; rest stay inf)
    # free + partition side both include the -0.5(|q|^2+|r|^2) bias,
    # so min d^2 = -2*max
    m2d = np.full((N, L, N), np.inf)
    for c in range(NCORES):
        outf = res.results[c]["outf"]          # [RUSE, 128, NQB]
        outp = res.results[c]["outp"]          # [RUSE, 1, LE]
        acco = res.results[c]["acco"]          # [128, LE] (last round's acc)
        for r in range(RUSE):
            a, b = sched[r][c]
            va = outf[r].transpose(1, 0).reshape(LP)[:L]   # q = qb*128+p
            m2d[a, :, b] = -2.0 * va.astype(np.float64)
            if r < RUSE - 1:
                vb = outp[r, 0, :L].astype(np.float64)
            else:
                vb = acco.astype(np.float64).max(axis=0)[:L]
            m2d[b, :, a] = -2.0 * vb
    if os.environ.get("KERNEL_DUMP"):
        np.save("/tmp/m2d_dev.npy", m2d)
    d = np.sqrt(np.maximum(m2d, 1e-12))
    for i in range(N):
        d[i, :, i] = np.inf
    kk = min(4, RUSE)   # mean of the kk smallest of the RUSE computed partners
    pscore = np.partition(d, kk - 1, axis=-1)[:, :, :kk].mean(-1)   # [N, L]

    img = _rescue(Z, pscore, trace)
    return _host_tail(img, cls_tokens)


def _rescue(Z, pscore, trace):
    if "nc2" not in _CACHE:
        _CACHE["nc2"] = _build2()
    nc2 = _CACHE["nc2"]

    cand = np.argsort(-pscore, axis=-1)[:, :NCAND]       # [16, 16]
    qidx = cand.reshape(-1)
    qimg = np.repeat(np.arange(N), NCAND)
    qf = Z[qimg, qidx].astype(np.float32)                # [256, 1024]
    qh = qf.astype(np.float16)
    # qc[h, p, k', cand] = qh[cand, 128(4h+k')+p] (streamed rhs, 2 k-halves)
    qct = np.ascontiguousarray(
        qh.reshape(NC_TOT, 2, KCH // 2, 128).transpose(1, 3, 2, 0))

    rh, rnb = _host_prep2(Z)
    in_maps = []
    for c in range(NCORES):
        sel = [2 * c, 2 * c + 1]
        in_maps.append({
            "qc": qct,
            "rh": np.ascontiguousarray(rh[sel]),
            "rnb": np.ascontiguousarray(rnb[sel]),
        })
    res2 = _run_with_retry(nc2, in_maps, trace)
    _CACHE["last_results2"] = res2

    v = np.zeros((NC_TOT, N))
    for c in range(NCORES):
        acco2 = res2.results[c]["acco2"]     # [2, 128, NC_TOT]
        for pos in range(2):
            v[:, 2 * c + pos] = acco2[pos].astype(np.float64).max(axis=0)
    q2c = (qf.astype(np.float64) ** 2).sum(-1)
    d2 = np.maximum(q2c[:, None] - 2.0 * v, 1e-12)
    dc = np.sqrt(d2)
    dc[np.arange(NC_TOT), qimg] = np.inf
    cscore = np.sort(dc, axis=-1)[:, :4].mean(-1)
    return cscore.reshape(N, NCAND).max(-1)


def _host_tail(img, cls_tokens):
    s = (img - img.min()) / (img.max() - img.min())
    W = cls_tokens.astype(np.float64) @ cls_tokens.astype(np.float64).T
    outs = []
    for k in (1, 2, 3):
        thr = np.sort(W, axis=-1)[:, N - k][:, None]
        Wm = np.where(W >= thr, W, 0.0)
        P = Wm / Wm.sum(-1, keepdims=True)
        outs.append(P @ s)
    return np.stack(outs, -1).mean(-1).astype(np.float32)



# revision 51
# speedup vs baseline: 1.1543x; 1.1543x over previous
"""MuSc (Mutual Scoring) Trainium2 kernel — v3 (truncated rounds + wide rescue).

Problem: nn_BatchMuSc — Z:[16,1369,1024] patch features, cls_tokens:[16,1024].
MSM: for each image i, per-patch score = mean of the 4 smallest per-image
min-distances (excluding self). Then image scores -> min-max norm -> MMO over
cls-token similarity.

Key insight: phase 1's ONLY job is selecting rescue candidates — the final
image scores come entirely from the exact fp16 rescue (phase 2). So phase 1
can be aggressively approximate:
  - RUSE=2 of the 15 pair-rounds are computed (each image scored against 2
    partners, statistic = mean of those 2 min-distances). Validated on host
    with exact distances: top-32 candidate selection recovers the argmax
    patch with 0/1280 failures even at 2x the empirical fp8 noise level
    (sigma_emp = 4.15 on d^2, worst model rank 23); on-device worst-case
    argmax rank is 15 of 32.
  - SYMMETRY: each unordered image pair {a,b} is computed ONCE as a
    [1408q x 1408r] block and reduced along BOTH axes: free-axis max ->
    a's patches vs b; partition-axis max (GpSimd all-reduce) -> b's patches
    vs a. 2 rounds x 8 cores = 16 pairs (1-factorization of K16).
  - fp8 e4m3 + DoubleRow matmuls (K=256 per MM): PE runs at the 157 TF/s fp8
    roofline, ~98% window utilization.
  - Both norms ride in the stream: ref-side -0.5|r|^2 as base-{64,8,1} fp8
    digit rows (feature rows 1021-1023 repurposed), query-side -0.5|q|^2 as
    the f32 ACT bias during the psum->f16 staging copy. So staged s2 =
    q.r - 0.5|r|^2 - 0.5|q|^2 and min d^2 = -2*max on both axes.
  - Per query block: 12 DR matmuls -> 2 ACT staging copies (psum->s2 f16,
    +bias) -> DVE acc=max(acc,s2) (partition side) -> DVE tensor_reduce
    (free side). Reading the staged f16 instead of f32 PSUM keeps DVE off
    the critical path.
  - DMA: k-subtile-major contiguous DRAM slabs; round-0 subtile split lets
    the first matmul start after ~700KB instead of 2.8MB.
  - Phase 2 (exact rescue): top-32 patches per image = 512 candidates,
    scored against ALL 15 other images at fp16, sharded over ref images
    (2 per core). Refs are the stationary operand (psum[ref, cand]); the
    candidates stream as one 512-wide rhs; the exact f32 ref norm rides as
    the ACT bias during psum->f32 staging; DVE maxes across j-blocks and
    the host does the final partition max. No aug matmuls, no on-device
    partition reduce. Host does the tiny 4-smallest/MMO tail in f64.
"""

import os
import numpy as np
import ml_dtypes

N = 16            # images
L = 1369          # patches per image
C = 1024          # feature dim
NCORES = 8
LP = 1408         # padded patches (11 * 128)
NQB = 11          # query blocks of 128
KCH = 8           # contraction chunks of 128
NSP = KCH // 2    # k-subtiles of 2 chunks (one fp8-DR matmul's contraction)
R = 15            # full pair-round count (1-factorization of K16)
RUSE = 2          # rounds actually computed: each image scored against 2
                  # partners; candidate top-32 is then exact-rescued against
                  # all 15 (see module docstring for the validation)
CHUNKS = [(0, 512), (512, 512), (1024, 352)]   # covers 1369 real + 7 pad cols
LE = 1376         # elementwise width (mult of 32)
PAD_VAL = 2.0     # pad-row feature value; pad distances are huge -> never win
BIG = 3.0e38
NTERM = 1         # rescue query terms (qh only; fp16 precision suffices)
NT2 = KCH * NTERM  # rescue lhsT slots
NCAND = 32        # rescued candidates per image (512 streamed columns)
NSLAB = N * NCAND // 128

_CACHE = {}


def _pair_schedule():
    """sched[r][c] = (a, b): round-robin 1-factorization of K16."""
    sched = []
    for r in range(R):
        pairs = [(15, r)]
        for k in range(1, 8):
            pairs.append(((r + k) % R, (r - k) % R))
        sched.append(pairs)
    return sched


def _build1(fp8=True, rounds=R):
    import concourse.bacc as bacc
    import concourse.tile as tile
    from concourse import mybir, bass_isa

    f16 = mybir.dt.float16
    f32 = mybir.dt.float32
    dt_z = mybir.dt.float8e4 if fp8 else f16
    Alu = mybir.AluOpType
    Copy = mybir.ActivationFunctionType.Copy
    DR = mybir.MatmulPerfMode.DoubleRow

    nc = bacc.Bacc("TRN2", target_bir_lowering=False, debug=False)

    # k-subtile-major DRAM layout: zta[r, s] is one contiguous [128, 2, LP] slab
    zta = nc.dram_tensor("zta", [rounds, NSP, 128, 2, LP], dt_z, kind="ExternalInput").ap()
    ztb = nc.dram_tensor("ztb", [rounds, NSP, 128, 2, LP], dt_z, kind="ExternalInput").ap()
    qna = nc.dram_tensor("qna", [rounds, 128, NQB], f32, kind="ExternalInput").ap()
    outf = nc.dram_tensor("outf", [rounds, 128, NQB], f32, kind="ExternalOutput").ap()
    outp = nc.dram_tensor("outp", [rounds, 1, LE], f32, kind="ExternalOutput").ap()
    # last round's raw accumulator: host does its partition max (saves the
    # exposed 4.8us GpSimd all-reduce on the tail)
    acco = nc.dram_tensor("acco", [128, LE], f16, kind="ExternalOutput").ap()

    Identity = mybir.ActivationFunctionType.Identity

    with tile.TileContext(nc) as tc:
        with (
            tc.tile_pool(name="zpool", bufs=2) as zpool,
            tc.tile_pool(name="qnpool", bufs=2) as qnpool,
            tc.tile_pool(name="accpool", bufs=2) as accpool,
            tc.tile_pool(name="s2pool", bufs=2) as s2pool,
            tc.tile_pool(name="outfpool", bufs=2) as outfpool,
            tc.tile_pool(name="prpool", bufs=2) as prpool,
            tc.tile_pool(name="psumA", bufs=3, space="PSUM") as psumA,
            tc.tile_pool(name="psumB", bufs=2, space="PSUM") as psumB,
        ):
            for r in range(rounds):
                # per-ksub DMA split: the first matmul only needs subtile 0
                # of both operands (~700KB) instead of the full 2.8MB; qn's
                # 128 tiny 44B descriptors go BEHIND subtile 0 (qn is not
                # consumed until the first ACT, ~2us after matmuls start)
                qn = qnpool.tile([128, NQB], f32, name="qn", tag="qn")
                za_s, zb_s = [], []
                for s in range(NSP):
                    ta = zpool.tile([128, 2, LP], dt_z, name=f"za{s}", tag=f"za{s}")
                    nc.sync.dma_start(ta[:], zta[r, s])
                    za_s.append(ta)
                    tb = zpool.tile([128, 2, LP], dt_z, name=f"zb{s}", tag=f"zb{s}")
                    nc.sync.dma_start(tb[:], ztb[r, s])
                    zb_s.append(tb)
                    if s == 0:
                        nc.sync.dma_start(qn[:], qna[r])

                acc = accpool.tile([128, LE], f16, name="acc", tag="acc")
                outf_t = outfpool.tile([128, NQB], f32, name="outf_t", tag="outf_t")

                for qb in range(NQB):
                    ptA = psumA.tile([128, 1024], f32, name="ptA", tag="ptA")
                    ptB = psumB.tile([128, 512], f32, name="ptB", tag="ptB")
                    for ci, (c0, w) in enumerate(CHUNKS):
                        dst = ptA[:, c0:c0 + w] if ci < 2 else ptB[:, :w]
                        if fp8:
                            for kp in range(NSP):
                                nc.tensor.matmul(
                                    dst,
                                    lhsT=za_s[kp][:, :, qb * 128:(qb + 1) * 128],
                                    rhs=zb_s[kp][:, :, c0:c0 + w],
                                    start=(kp == 0),
                                    stop=(kp == NSP - 1),
                                    perf_mode=DR,
                                )
                        else:
                            for k in range(KCH):
                                nc.tensor.matmul(
                                    dst,
                                    lhsT=za_s[k // 2][:, k % 2, qb * 128:(qb + 1) * 128],
                                    rhs=zb_s[k // 2][:, k % 2, c0:c0 + w],
                                    start=(k == 0),
                                    stop=(k == KCH - 1),
                                )
                    # partition side staging: s2 = psum + (-0.5|q|^2), fp16;
                    # qb 0 writes the accumulator directly
                    if qb == 0:
                        s2 = acc
                    else:
                        s2 = s2pool.tile([128, LE], f16, name="s2", tag="s2")
                    nc.scalar.activation(
                        s2[:, :1024], ptA[:, :1024], Identity,
                        bias=qn[:, qb:qb + 1], scale=1.0)
                    nc.scalar.activation(
                        s2[:, 1024:LE], ptB[:, :352], Identity,
                        bias=qn[:, qb:qb + 1], scale=1.0)
                    # accmax first so the final round's partition reduce can
                    # start while the last free-side reduce still runs
                    if qb > 0:
                        nc.vector.tensor_tensor(
                            acc[:, :LE], acc[:, :LE], s2[:, :LE], op=Alu.max)
                    # free side: max over refs from the f16 staged copy; outf
                    # includes the -0.5|q|^2 bias, so host uses -2*outf
                    nc.vector.tensor_reduce(
                        outf_t[:, qb:qb + 1], s2[:, :LE],
                        axis=mybir.AxisListType.X, op=Alu.max)

                if r < rounds - 1:
                    pr = prpool.tile([128, LE], f32, name="pr", tag="pr")
                    nc.gpsimd.partition_all_reduce(
                        pr[:], acc[:, :LE], channels=128,
                        reduce_op=bass_isa.ReduceOp.max)
                    nc.sync.dma_start(outp[r], pr[0:1, :])
                else:
                    nc.sync.dma_start(acco[:], acc[:, :LE])
                nc.sync.dma_start(outf[r], outf_t[:])
    nc.compile()
    return nc


NC_TOT = N * NCAND          # 256 rescue candidates
WAVES = [(0, 6), (6, NQB)]  # j-blocks per PSUM wave (6 + 5 one-bank tiles)


def _build2():
    """Exact rescue v2: refs stationary, candidates streamed.

    Per (pos, j-block): psum[ref, cand] = sum_k rk[k][:, jblock].T @ qct[k]
    (fp16, K=1024 over 8 matmuls, 256 streamed cols each). The exact ref norm
    -0.5|r|^2 rides as the per-partition f32 ACT bias during psum->SBUF f32
    staging; DVE maxes the staged blocks across j; the host does the final
    partition max (no on-device partition reduce, no aug matmuls). k-outer
    loop within each wave so the first matmuls need only qct[0]+rk[0]
    (~425KB) instead of the full 2.9MB."""
    import concourse.bacc as bacc
    import concourse.tile as tile
    from concourse import mybir

    f16 = mybir.dt.float16
    f32 = mybir.dt.float32
    Alu = mybir.AluOpType
    Identity = mybir.ActivationFunctionType.Identity

    nc = bacc.Bacc("TRN2", target_bir_lowering=False, debug=False)
    # two contiguous k-half candidate tensors (4KB rows each; finer per-k
    # tiles would DMA tiny descriptors and clog the queues ahead of the
    # refs, one full tile would delay the first matmul by ~1.5us)
    qc = nc.dram_tensor("qc", [2, 128, KCH // 2, NC_TOT], f16, kind="ExternalInput").ap()
    rh = nc.dram_tensor("rh", [2, KCH, 128, LP], f16, kind="ExternalInput").ap()
    rnb = nc.dram_tensor("rnb", [2, 128, NQB], f32, kind="ExternalInput").ap()
    acco2 = nc.dram_tensor("acco2", [2, 128, NC_TOT], f32, kind="ExternalOutput").ap()

    with tile.TileContext(nc) as tc:
        with (
            tc.tile_pool(name="qpool2", bufs=1) as qpool2,
            tc.tile_pool(name="ref2", bufs=2) as ref2,
            tc.tile_pool(name="rnpool", bufs=2) as rnpool,
            tc.tile_pool(name="acc2pool", bufs=2) as acc2pool,
            tc.tile_pool(name="s3pool", bufs=3) as s3pool,
            tc.tile_pool(name="ps2", bufs=6, space="PSUM") as ps2,
        ):
            qct = [None] * KCH

            def _load_qhalf(h):
                qtile = qpool2.tile([128, KCH // 2, NC_TOT], f16,
                                    name=f"qtile{h}", tag=f"qtile{h}")
                nc.sync.dma_start(qtile[:], qc[h])
                for kk_ in range(KCH // 2):
                    qct[h * (KCH // 2) + kk_] = qtile[:, kk_, :]

            _load_qhalf(0)
            for pos in range(2):
                rnbt = rnpool.tile([128, NQB], f32, name="rnbt", tag="rnbt")
                nc.sync.dma_start(rnbt[:], rnb[pos])
                rkt = []
                for k in range(KCH):
                    t_ = ref2.tile([128, LP], f16, name=f"rk{k}", tag=f"rk{k}")
                    nc.sync.dma_start(t_[:], rh[pos, k])
                    rkt.append(t_)
                    if pos == 0 and k == 0:
                        # second candidate half rides behind the first ref
                        # tile: the k=0 matmuls only need half 0
                        _load_qhalf(1)

                acc2 = acc2pool.tile([128, NC_TOT], f32, name="acc2", tag="acc2")
                for w0, w1 in WAVES:
                    pt = {}
                    for j in range(w0, w1):
                        # one full bank per j (256 f32 used, bank-aligned dst)
                        pt[j] = ps2.tile([128, 512], f32, name=f"pt{j}", tag="pt")
                    for k in range(KCH):
                        for j in range(w0, w1):
                            nc.tensor.matmul(
                                pt[j][:, :NC_TOT],
                                lhsT=rkt[k][:, j * 128:(j + 1) * 128],
                                rhs=qct[k][:],
                                start=(k == 0),
                                stop=(k == KCH - 1),
                            )
                    for j in range(w0, w1):
                        if j == 0:
                            s3 = acc2
                        else:
                            s3 = s3pool.tile([128, NC_TOT], f32, name="s3", tag="s3")
                        nc.scalar.activation(
                            s3[:, :NC_TOT], pt[j][:, :NC_TOT], Identity,
                            bias=rnbt[:, j:j + 1], scale=1.0)
                        if j > 0:
                            nc.vector.tensor_tensor(
                                acc2[:], acc2[:], s3[:, :NC_TOT], op=Alu.max)
                nc.sync.dma_start(acco2[pos], acc2[:])
    nc.compile()
    return nc


DIGIT_SCALES = (64.0, 8.0, 1.0)


def _digit_rows(v):
    """Decompose v (~[-2100, -400]) into base-{64,8,1} rows, last row e4m3."""
    d1 = np.round(v / 64.0)
    r1 = v - 64.0 * d1
    d2 = np.round(r1 / 8.0)
    d3 = r1 - 8.0 * d2
    return d1, d2, d3


def _host_prep(Z, fp8=True):
    """Quantized transposed tiles (a/b variants) + exact norms + qn bias.

    Feature rows 1021-1023 (p=125..127 of k-chunk 7) are repurposed:
    a-variant (lhsT) holds the constants {64, 8, 1}; b-variant (rhs) holds
    the base-{64,8,1} digit rows of -0.5|r|^2, so the DR stream itself
    computes q.r(1021 feats) - 0.5|r|^2.
    """
    Zp = np.full((N, LP, C), PAD_VAL, dtype=np.float32)
    Zp[:, :L, :] = Z
    qdt = ml_dtypes.float8_e4m3 if fp8 else np.float16
    Zq = Zp.astype(qdt)
    # [img, p, k, r] = Zq[img, r, 128k+p]
    zt = np.ascontiguousarray(Zq.reshape(N, LP, KCH, 128).transpose(0, 3, 2, 1))
    nrm = (Zp.astype(np.float64) ** 2).sum(-1)          # [N, LP] exact full norm
    zta = zt.copy()
    for j, s in enumerate(DIGIT_SCALES):
        zta[:, 125 + j, 7, :] = qdt(s)
    ztb = zt
    d1, d2, d3 = _digit_rows(-0.5 * nrm)
    ztb[:, 125, 7, :] = d1.astype(qdt)
    ztb[:, 126, 7, :] = d2.astype(qdt)
    ztb[:, 127, 7, :] = d3.astype(qdt)
    qna = np.ascontiguousarray(
        (-0.5 * nrm).astype(np.float32).reshape(N, NQB, 128).transpose(0, 2, 1))

    def ksub_major(zt_):
        # [img, p, k, l] -> [img, s, p, j, l] with k = 2s+j (contiguous
        # per-(img,s) slabs for large-descriptor DMA)
        t = zt_.transpose(0, 2, 1, 3).reshape(N, NSP, 2, 128, LP)
        return np.ascontiguousarray(t.transpose(0, 1, 3, 2, 4))

    return ksub_major(zta), ksub_major(ztb), nrm, qna


def _host_prep2(Z):
    """Rescue ref data: fp16 refs (k-major contiguous slabs) + exact f32
    -0.5|r|^2 bias in phase-1 qna layout ([img, p, j] for ref patch 128j+p)."""
    Zp = np.full((N, LP, C), PAD_VAL, dtype=np.float32)
    Zp[:, :L, :] = Z
    Zh = Zp.astype(np.float16)
    # [img, k, p, l] = Zh[img, l, 128k+p]  (rh[pos, k] contiguous)
    rh = np.ascontiguousarray(Zh.reshape(N, LP, KCH, 128).transpose(0, 2, 3, 1))
    nrm = (Zp.astype(np.float64) ** 2).sum(-1)
    rnb = np.ascontiguousarray(
        (-0.5 * nrm).astype(np.float32).reshape(N, NQB, 128).transpose(0, 2, 1))
    return rh, rnb


def _run_with_retry(nc, in_maps, trace, attempts=3):
    import time
    import traceback
    import concourse.bass_utils as bass_utils

    import jax
    jax.devices()   # force PJRT backend init before the NTFF profile hook

    for a in range(attempts):
        try:
            return bass_utils.run_bass_kernel_spmd(
                nc, in_maps, core_ids=list(range(NCORES)), trace=trace)
        except Exception:
            traceback.print_exc()
            if a == attempts - 1:
                raise
            time.sleep(5)


def kernel(Z, cls_tokens):
    Z = np.asarray(Z, dtype=np.float32)
    cls_tokens = np.asarray(cls_tokens)
    fp8 = bool(int(os.environ.get("KERNEL_FP8", "1")))
    trace = bool(int(os.environ.get("KERNEL_TRACE", "0")))

    if "nc1" not in _CACHE:
        _CACHE["nc1"] = _build1(fp8=fp8, rounds=RUSE)
    nc1 = _CACHE["nc1"]

    zta_all, ztb_all, nrm, qna = _host_prep(Z, fp8=fp8)
    sched = _pair_schedule()

    in_maps = []
    for c in range(NCORES):
        aa = [sched[r][c][0] for r in range(RUSE)]
        bb = [sched[r][c][1] for r in range(RUSE)]
        in_maps.append({
            "zta": np.ascontiguousarray(zta_all[aa]),
            "ztb": np.ascontiguousarray(ztb_all[bb]),
            "qna": np.ascontiguousarray(qna[aa]),
        })

    res = _run_with_retry(nc1, in_maps, trace)
    _CACHE["last_results"] = res

    # assemble per-patch min-d^2 matrix [img, patch, other-img]
    # (only RUSE of 15 partners computed; rest stay inf)
    # free + partition side both include the -0.5(|q|^2+|r|^2) bias,
    # so min d^2 = -2*max
    m2d = np.full((N, L, N), np.inf)
    for c in range(NCORES):
        outf = res.results[c]["outf"]          # [RUSE, 128, NQB]
        outp = res.results[c]["outp"]          # [RUSE, 1, LE]
        acco = res.results[c]["acco"]          # [128, LE] (last round's acc)
        for r in range(RUSE):
            a, b = sched[r][c]
            va = outf[r].transpose(1, 0).reshape(LP)[:L]   # q = qb*128+p
            m2d[a, :, b] = -2.0 * va.astype(np.float64)
            if r < RUSE - 1:
                vb = outp[r, 0, :L].astype(np.float64)
            else:
                vb = acco.astype(np.float64).max(axis=0)[:L]
            m2d[b, :, a] = -2.0 * vb
    if os.environ.get("KERNEL_DUMP"):
        np.save("/tmp/m2d_dev.npy", m2d)
    d = np.sqrt(np.maximum(m2d, 1e-12))
    for i in range(N):
        d[i, :, i] = np.inf
    kk = min(4, RUSE)   # mean of the kk smallest of the RUSE computed partners
    pscore = np.partition(d, kk - 1, axis=-1)[:, :, :kk].mean(-1)   # [N, L]

    img = _rescue(Z, pscore, trace)
    return _host_tail(img, cls_tokens)


def _rescue(Z, pscore, trace):
    if "nc2" not in _CACHE:
        _CACHE["nc2"] = _build2()
    nc2 = _CACHE["nc2"]

    cand = np.argsort(-pscore, axis=-1)[:, :NCAND]       # [16, 16]
    qidx = cand.reshape(-1)
    qimg = np.repeat(np.arange(N), NCAND)
    qf = Z[qimg, qidx].astype(np.float32)                # [256, 1024]
    qh = qf.astype(np.float16)
    # qc[h, p, k', cand] = qh[cand, 128(4h+k')+p] (streamed rhs, 2 k-halves)
    qct = np.ascontiguousarray(
        qh.reshape(NC_TOT, 2, KCH // 2, 128).transpose(1, 3, 2, 0))

    rh, rnb = _host_prep2(Z)
    in_maps = []
    for c in range(NCORES):
        sel = [2 * c, 2 * c + 1]
        in_maps.append({
            "qc": qct,
            "rh": np.ascontiguousarray(rh[sel]),
            "rnb": np.ascontiguousarray(rnb[sel]),
        })
    res2 = _run_with_retry(nc2, in_maps, trace)
    _CACHE["last_results2"] = res2

    v = np.zeros((NC_TOT, N))
    for c in range(NCORES):
        acco2 = res2.results[c]["acco2"]     # [2, 128, NC_TOT]
        for pos in range(2):
            v[:, 2 * c + pos] = acco2[pos].astype(np.float64).max(axis=0)
    q2c = (qf.astype(np.float64) ** 2).sum(-1)
    d2 = np.maximum(q2c[:, None] - 2.0 * v, 1e-12)
    dc = np.sqrt(d2)
    dc[np.arange(NC_TOT), qimg] = np.inf
    cscore = np.sort(dc, axis=-1)[:, :4].mean(-1)
    return cscore.reshape(N, NCAND).max(-1)


def _host_tail(img, cls_tokens):
    s = (img - img.min()) / (img.max() - img.min())
    W = cls_tokens.astype(np.float64) @ cls_tokens.astype(np.float64).T
    outs = []
    for k in (1, 2, 3):
        thr = np.sort(W, axis=-1)[:, N - k][:, None]
        Wm = np.where(W >= thr, W, 0.0)
        P = Wm / Wm.sum(-1, keepdims=True)
        outs.append(P @ s)
    return np.stack(outs, -1).mean(-1).astype(np.float32)

